# revision 2
# baseline (speedup 1.0000x reference)
"""Trainium2 Bass kernel for LowRankRayTracer.

csi[f] = (delta_t/D) * v_f^T M v_f,  M = conj(rad)^T conj(att)  (R=32, complex)
contracted over N = D*K = 524288 rows.

Strategy (8 cores):
  - Shard the N rows across cores (512 directions each). csi is linear in M,
    so each core computes its partial S = rad32^T att32 (64x64, f32 view of
    complex pairs -> all four real cross products at once), builds
    W = [W_real | W_imag] (block form), computes partial csi over ALL F=8192
    subcarriers, and the host just sums the 8 partial csi vectors.
  - fp32 matmul is 4 cyc/col on TRN2 PE, so inputs are split on the host into
    fp16 hi+lo (same total bytes); with the att hi/lo pair packed side by side
    as one 256-wide moving operand, two matmuls per slice (lhsT=rad_h, rad_l)
    produce all four products hh|hl|lh|ll -- exact reconstruction, and half
    the LDWEIGHTS of a 3-pass version (LDW is the PE bottleneck otherwise).
  - Matmuls accumulate round-robin into 4 PSUM banks (avoids same-bank RMW
    serialization); diagonal blocks summed later via selection matmuls.
"""

import numpy as np

D, K, R = 4096, 128, 32
F = 8192
N_CORES = 8
DIR_PER_CORE = D // N_CORES              # 512
ROWS_PER_CORE = DIR_PER_CORE * K         # 65536 rows of (64,) f32
N_MACRO = 8                              # macro tiles per tensor per core
MACRO_COLS = 4096                        # fp16 per partition per macro tile
SLICE = 128                              # matmul slice width (2 rows/partition)
SCALE = (200.0 / K) / D                  # delta_t / num_directions (exact binary)
FCHUNK = 512                             # phase-3 subcarriers per chunk
N_FCHUNK = F // FCHUNK                   # 16
NB = 4                                   # round-robin PSUM accumulator banks

_NC_CACHE = {}


def _build_consts():
    """(128, 258) f32: four (128,64) selection matrices + ones-selector cols."""
    c = np.zeros((128, 258), np.float32)
    EA = np.zeros((128, 32), np.float32)
    OA = np.zeros((128, 32), np.float32)
    EB = np.zeros((128, 32), np.float32)
    OB = np.zeros((128, 32), np.float32)
    for m in range(32):
        EA[2 * m, m] = 1.0
        OA[2 * m + 1, m] = 1.0
        EB[64 + 2 * m, m] = 1.0
        OB[64 + 2 * m + 1, m] = 1.0
    c[:, 0:32] = EA
    c[:, 32:64] = OA
    c[:, 64:96] = EB
    c[:, 96:128] = OB
    c[:, 128:160] = OA
    c[:, 160:192] = EA
    c[:, 192:224] = OB
    c[:, 224:256] = EB
    c[0:64, 256] = 1.0
    c[64:128, 257] = 1.0
    return c


def build_nc(n_macro=N_MACRO):
    import concourse.bacc as bacc
    import concourse.mybir as mybir
    import concourse.tile as tile

    fp32 = mybir.dt.float32
    fp16 = mybir.dt.float16
    nc = bacc.Bacc(trn_type="TRN2", target_bir_lowering=False, debug=False)

    rad_h_d = nc.dram_tensor("rad_h", [n_macro, 128, MACRO_COLS], fp16,
                             kind="ExternalInput").ap()
    rad_l_d = nc.dram_tensor("rad_l", [n_macro, 128, MACRO_COLS], fp16,
                             kind="ExternalInput").ap()
    att_hl_d = nc.dram_tensor("att_hl", [n_macro, 128, 2 * MACRO_COLS], fp16,
                              kind="ExternalInput").ap()
    gtd_d = nc.dram_tensor("gtd", [128, F], fp32, kind="ExternalInput").ap()
    gth_d = nc.dram_tensor("gth", [64, F], fp16, kind="ExternalInput").ap()
    gtl_d = nc.dram_tensor("gtl", [64, F], fp16, kind="ExternalInput").ap()
    cst_d = nc.dram_tensor("consts", [128, 258], fp32, kind="ExternalInput").ap()
    out_d = nc.dram_tensor("csi", [2, F], fp32, kind="ExternalOutput").ap()

    with tile.TileContext(nc) as tc:
        with (
            tc.tile_pool(name="io", bufs=2) as io_pool,
            tc.tile_pool(name="small", bufs=1) as small,
            tc.tile_pool(name="epool", bufs=8) as epool,
        ):
            # constants up front (tiny); gtd issued after the main-loop DMAs
            # so it doesn't steal early HBM bandwidth (not needed till phase 3)
            c_sb = small.tile([128, 258], fp32, tag="consts")
            nc.sync.dma_start(c_sb[:], cst_d[:])
            gtd_sb = small.tile([128, F], fp32, tag="gtd")
            gth_sb = small.tile([64, F], fp16, tag="gth")
            gtl_sb = small.tile([64, F], fp16, tag="gtl")

            # ---- main loop: S += rad^T att via fp16 hi/lo, 256-wide rhs ----
            # lhsT=rad_h over rhs=[att_h|att_l] gives [hh|hl]; lhsT=rad_l
            # gives [lh|ll]. S = sum of all four 128-col blocks (exact).
            s_sb = small.tile([128, 128], fp32, tag="s_sb")
            n_slices = MACRO_COLS // SLICE
            total = n_macro * n_slices * 2
            with tc.tile_pool(name="spsum", bufs=1, space="PSUM") as spsum:
                banks = [spsum.tile([128, 2 * SLICE], fp32, tag=f"s{b}",
                                    name=f"sbank{b}")
                         for b in range(NB)]
                seen = [False] * NB
                idx = 0
                for i in range(n_macro):
                    rad_h = io_pool.tile([128, MACRO_COLS], fp16, tag="rad_h")
                    rad_l = io_pool.tile([128, MACRO_COLS], fp16, tag="rad_l")
                    att_hl = io_pool.tile([128, 2 * MACRO_COLS], fp16,
                                          tag="att_hl")
                    if i == 0:
                        # halve the first loads so the first matmuls start
                        # as soon as ~1.5 MiB has landed, not 4 MiB
                        hm = MACRO_COLS // 2
                        nc.sync.dma_start(rad_h[:, 0:hm], rad_h_d[0, :, 0:hm])
                        nc.scalar.dma_start(att_hl[:, 0:2 * hm],
                                            att_hl_d[0, :, 0:2 * hm])
                        nc.sync.dma_start(rad_l[:, 0:hm], rad_l_d[0, :, 0:hm])
                        nc.sync.dma_start(rad_h[:, hm:], rad_h_d[0, :, hm:])
                        nc.scalar.dma_start(att_hl[:, 2 * hm:],
                                            att_hl_d[0, :, 2 * hm:])
                        nc.sync.dma_start(rad_l[:, hm:], rad_l_d[0, :, hm:])
                    else:
                        nc.sync.dma_start(rad_h[:], rad_h_d[i, :, :])
                        nc.sync.dma_start(rad_l[:], rad_l_d[i, :, :])
                        nc.scalar.dma_start(att_hl[:], att_hl_d[i, :, :])
                    for s in range(n_slices):
                        rsl = slice(s * SLICE, (s + 1) * SLICE)
                        asl = slice(s * 2 * SLICE, (s + 1) * 2 * SLICE)
                        for lh in (rad_h, rad_l):
                            b = idx % NB
                            nc.tensor.matmul(
                                banks[b][:],
                                lhsT=lh[:, rsl],
                                rhs=att_hl[:, asl],
                                start=not seen[b],
                                stop=(idx >= total - NB),
                            )
                            seen[b] = True
                            idx += 1

                nc.sync.dma_start(gtd_sb[:], gtd_d[:])
                nc.sync.dma_start(gth_sb[:], gth_d[:])
                nc.sync.dma_start(gtl_sb[:], gtl_d[:])

                # S = sum of all four 128-col blocks over the 4 banks
                acc = small.tile([128, 2 * SLICE], fp32, tag="acc")
                nc.vector.tensor_copy(acc[:], banks[0][:])
                for b in range(1, NB):
                    nc.vector.tensor_add(acc[:], acc[:], banks[b][:])
                nc.vector.tensor_add(s_sb[:], acc[:, 0:SLICE],
                                     acc[:, SLICE:2 * SLICE])

            # ---- epilogue: build W = [W_real | W_imag] (64, 128) ----
            with tc.tile_pool(name="vpsum", bufs=1, space="PSUM") as vpsum:
                v1 = vpsum.tile([64, 64], fp32, tag="v1")
                nc.tensor.matmul(v1[:], lhsT=c_sb[:, 0:64], rhs=s_sb[:, 0:64],
                                 start=True, stop=False)
                nc.tensor.matmul(v1[:], lhsT=c_sb[:, 64:128],
                                 rhs=s_sb[:, 64:128], start=False, stop=True)
                v2 = vpsum.tile([64, 64], fp32, tag="v2")
                nc.tensor.matmul(v2[:], lhsT=c_sb[:, 128:192],
                                 rhs=s_sb[:, 0:64], start=True, stop=False)
                nc.tensor.matmul(v2[:], lhsT=c_sb[:, 192:256],
                                 rhs=s_sb[:, 64:128], start=False, stop=True)

                v1s = small.tile([64, 64], fp32, tag="v1s")
                nc.vector.tensor_copy(v1s[:], v1[:])
                v2s = small.tile([64, 64], fp32, tag="v2s")
                nc.vector.tensor_copy(v2s[:], v2[:])

            # mr = Mr (dup-stacked), mp = -Mi (dup-stacked)
            mr = small.tile([64, 32], fp32, tag="mr")
            mp = small.tile([64, 32], fp32, tag="mp")
            nc.vector.tensor_sub(mr[0:32, :], v1s[0:32, 0:64:2], v2s[0:32, 1:64:2])
            nc.vector.tensor_sub(mr[32:64, :], v2s[32:64, 0:64:2], v1s[32:64, 1:64:2])
            nc.vector.tensor_add(mp[0:32, :], v1s[0:32, 1:64:2], v2s[0:32, 0:64:2])
            nc.vector.tensor_add(mp[32:64, :], v2s[32:64, 1:64:2], v1s[32:64, 0:64:2])

            wri = small.tile([64, 128], fp32, tag="wri")
            s_ = float(SCALE)
            # W_real = [[Mr, -Mi], [-Mi, -Mr]] * s
            nc.scalar.mul(wri[0:32, 0:32], mr[0:32, :], s_)
            nc.scalar.mul(wri[0:32, 32:64], mp[0:32, :], s_)
            nc.scalar.mul(wri[32:64, 0:32], mp[32:64, :], s_)
            nc.scalar.mul(wri[32:64, 32:64], mr[32:64, :], -s_)
            # W_imag = [[Mi, Mr], [Mr, -Mi]] * s
            nc.scalar.mul(wri[0:32, 64:96], mp[0:32, :], -s_)
            nc.scalar.mul(wri[0:32, 96:128], mr[0:32, :], s_)
            nc.scalar.mul(wri[32:64, 64:96], mr[32:64, :], s_)
            nc.scalar.mul(wri[32:64, 96:128], mp[32:64, :], s_)

            # fp16 hi/lo split of W for the phase-3 matmuls
            wh = small.tile([64, 128], fp16, tag="wh")
            nc.vector.tensor_copy(wh[:], wri[:])
            whf = small.tile([64, 128], fp32, tag="whf")
            nc.vector.tensor_copy(whf[:], wh[:])
            wlf = small.tile([64, 128], fp32, tag="wlf")
            nc.vector.tensor_sub(wlf[:], wri[:], whf[:])
            wl = small.tile([64, 128], fp16, tag="wl")
            nc.vector.tensor_copy(wl[:], wlf[:])

            # PE warm-keepers: cheap matmuls dependent on s_sb bridge the
            # epilogue gap so HAM doesn't re-throttle before phase 3
            with tc.tile_pool(name="wpsum", bufs=1, space="PSUM") as wpsum:
                warm_ps = wpsum.tile([64, 64], fp32, tag="warm")
                for w in range(10):
                    nc.tensor.matmul(warm_ps[:], lhsT=c_sb[:, 0:64],
                                     rhs=s_sb[:, 0:64], start=True, stop=True)

            # ---- phase 3: csi chunks over F ----
            # All T matmuls issued first so the per-chunk csi matmuls don't
            # head-of-line-block them in the in-order PE queue.
            csi_sb = small.tile([2, F], fp32, tag="csi_sb")
            with (
                tc.tile_pool(name="tpsum", bufs=6, space="PSUM") as tpsum,
                tc.tile_pool(name="cpsum", bufs=2, space="PSUM") as cpsum,
            ):
                t_tiles = []
                e_tiles = []
                for ci in range(N_FCHUNK):
                    fs = slice(ci * FCHUNK, (ci + 1) * FCHUNK)
                    t_ps = tpsum.tile([128, FCHUNK], fp32, tag="t",
                                      name=f"t{ci}")
                    # T = W^T g via fp16 hi/lo (dropped Wl*gl ~ 2^-22)
                    nc.tensor.matmul(t_ps[:], lhsT=wh[:], rhs=gth_sb[:, fs],
                                     start=True, stop=False)
                    nc.tensor.matmul(t_ps[:], lhsT=wl[:], rhs=gth_sb[:, fs],
                                     start=False, stop=False)
                    nc.tensor.matmul(t_ps[:], lhsT=wh[:], rhs=gtl_sb[:, fs],
                                     start=False, stop=True)
                    t_tiles.append(t_ps)
                    e_sb = epool.tile([128, FCHUNK], fp32, tag="e",
                                      name=f"e{ci}")
                    nc.vector.tensor_mul(e_sb[:], gtd_sb[:, fs], t_ps[:])
                    e_tiles.append(e_sb)
                for ci in range(N_FCHUNK):
                    fs = slice(ci * FCHUNK, (ci + 1) * FCHUNK)
                    c_ps = cpsum.tile([2, FCHUNK], fp32, tag="c",
                                      name=f"c{ci}")
                    nc.tensor.matmul(c_ps[:], lhsT=c_sb[:, 256:258],
                                     rhs=e_tiles[ci][:], start=True, stop=True)
                    nc.scalar.copy(csi_sb[:, fs], c_ps[:])

            nc.sync.dma_start(out_d[:], csi_sb[:])

    nc.compile()
    return nc


def _prep_shared(fbv):
    """gtd (128,F) f32 dup + fp16 hi/lo (64,F) from complex fbv (F, R)."""
    fbv32 = np.ascontiguousarray(fbv).view(np.float32).reshape(F, 2 * R)
    gbt = np.ascontiguousarray(
        np.concatenate([fbv32[:, 0::2].T, fbv32[:, 1::2].T], axis=0))
    gtd = np.ascontiguousarray(np.concatenate([gbt, gbt], axis=0))
    gth = gbt.astype(np.float16)
    gtl = (gbt - gth.astype(np.float32)).astype(np.float16)
    return gtd, gth, gtl


def _shard_hl(arr, core):
    """Core's complex64 shard -> (hi, lo) fp16 arrays (N_MACRO,128,MACRO_COLS)."""
    sh = arr[core * DIR_PER_CORE:(core + 1) * DIR_PER_CORE]
    f32 = np.ascontiguousarray(sh).view(np.float32).ravel()
    h = f32.astype(np.float16)
    lo = (f32 - h.astype(np.float32)).astype(np.float16)
    shp = (N_MACRO, 128, MACRO_COLS)
    return h.reshape(shp), lo.reshape(shp)


def _pack_hl(h, lo):
    """Interleave hi/lo at 128-col slice granularity: [...,s*256:+256] =
    [h_slice(128) | lo_slice(128)] -> (N_MACRO, 128, 2*MACRO_COLS)."""
    ns = MACRO_COLS // SLICE
    h4 = h.reshape(N_MACRO, 128, ns, SLICE)
    l4 = lo.reshape(N_MACRO, 128, ns, SLICE)
    return np.ascontiguousarray(
        np.stack([h4, l4], axis=3).reshape(N_MACRO, 128, 2 * MACRO_COLS))


def _build_in_maps(attenuation_vectors, radiation_vectors,
                   frequency_basis_vectors):
    gtd, gth, gtl = _prep_shared(frequency_basis_vectors)
    consts = _build_consts()
    in_maps = []
    for c in range(N_CORES):
        rh, rl = _shard_hl(radiation_vectors, c)
        ah, al = _shard_hl(attenuation_vectors, c)
        in_maps.append({
            "rad_h": rh, "rad_l": rl,
            "att_hl": _pack_hl(ah, al),
            "gtd": gtd, "gth": gth, "gtl": gtl,
            "consts": consts,
        })
    return in_maps


def kernel(attenuation_vectors, radiation_vectors, frequency_basis_vectors):
    from concourse.bass_utils import run_bass_kernel_spmd

    if "nc" not in _NC_CACHE:
        _NC_CACHE["nc"] = build_nc()
    nc = _NC_CACHE["nc"]

    in_maps = _build_in_maps(attenuation_vectors, radiation_vectors,
                             frequency_basis_vectors)
    res = run_bass_kernel_spmd(nc, in_maps, core_ids=list(range(N_CORES)))
    acc = np.zeros((2, F), np.float64)
    for r in res.results:
        acc += r["csi"]
    return (acc[0] + 1j * acc[1]).astype(np.complex64)



# revision 3
# speedup vs baseline: 2.0257x; 2.0257x over previous
"""Trainium2 Bass kernel for LowRankRayTracer.

csi[f] = (delta_t/D) * v_f^T M v_f,  M = conj(rad)^T conj(att)  (R=32, complex)
contracted over N = D*K = 524288 rows.

v2 design (8 cores, ray-sharded, all-fp16 data path):
  - Host converts each core's ray shard to PLANAR fp16 rows u = [Re|Im] (64
    wide), 2-packed per partition: tile (128, 4096) x 8, so each 128-col slice
    is one lhsT/rhs pair contracting 256 rows.  fp16 halves HBM traffic vs the
    exact hi/lo fp32 split; quantization error ~5e-4 << 2e-2 gate.
  - 256 matmuls accumulate S (128x128 quadrants) into 2 PSUM banks.
  - W_real = [[Q,P],[P,-Q]] (Q=S_rr-S_ii, P=S_ri+S_ir, scale folded into
    selector consts) is built by 8 small selector matmuls straight into PSUM,
    then cast twice into whh = [W|W] (64x128 fp16).  Because
    T_imag = rot(T_real), one T = whh^T g matmul per chunk covers re+im.
  - e = g_stack .* T (g_stack rows: vr,vi,vi,-vr) via DVE (direct or
    ACT-cast+DVE-2x split to balance engines), then per-chunk selector
    matmuls accumulate all 16 chunk sums into ONE (32,512) PSUM tile:
    a single PSUM->SBUF copy + one 64 KiB output DMA.
"""

import numpy as np

D, K, R = 4096, 128, 32
F = 8192
N_CORES = 8
DIR_PER_CORE = D // N_CORES              # 512
N_TILE = 8                               # DMA tiles per tensor per core
TILE_COLS = 4096                         # fp16 cols per partition per tile
N_SLICE = TILE_COLS // 128               # 32 matmul slices per tile
NB = 2                                   # round-robin PSUM accumulator banks
S_SCALE = (200.0 / K) / D                # delta_t / num_directions (exact)
FCHUNK = 512
N_FCHUNK = F // FCHUNK                   # 16

_NC_CACHE = {}


def _build_sel():
    """(128, 384) fp16: C_IA|C_IB|C_XA|C_XB|-C_XA|-C_XB, scale folded in."""
    s = np.float16(S_SCALE)
    c = np.zeros((128, 384), np.float16)
    for p in range(64):
        c[p, p] = s            # C_IA
        c[64 + p, 64 + p] = s  # C_IB
    for i in range(32):
        c[i, 128 + 32 + i] = s       # C_XA
        c[32 + i, 128 + i] = -s
        c[64 + i, 192 + 32 + i] = s  # C_XB
        c[96 + i, 192 + i] = -s
    c[:, 256:320] = -c[:, 128:192]   # -C_XA
    c[:, 320:384] = -c[:, 192:256]   # -C_XB
    return c


def _build_csel():
    """(128, 512) fp16: chunk ci block (128,32) sums rows 0:64 -> col 2ci,
    rows 64:128 -> col 2ci+1."""
    c = np.zeros((128, 512), np.float16)
    for ci in range(N_FCHUNK):
        c[0:64, 32 * ci + 2 * ci] = 1.0
        c[64:128, 32 * ci + 2 * ci + 1] = 1.0
    return c


def build_nc():
    import concourse.bacc as bacc
    import concourse.mybir as mybir
    import concourse.tile as tile

    fp32 = mybir.dt.float32
    fp16 = mybir.dt.float16
    nc = bacc.Bacc(trn_type="TRN2", target_bir_lowering=False, debug=False)

    rad_d = nc.dram_tensor("rad", [N_TILE, 128, TILE_COLS], fp16,
                           kind="ExternalInput").ap()
    att_d = nc.dram_tensor("att", [N_TILE, 128, TILE_COLS], fp16,
                           kind="ExternalInput").ap()
    gs_d = nc.dram_tensor("gs", [128, F], fp16, kind="ExternalInput").ap()
    sel_d = nc.dram_tensor("sel", [128, 384], fp16, kind="ExternalInput").ap()
    csel_d = nc.dram_tensor("csel", [128, 512], fp16,
                            kind="ExternalInput").ap()
    out_d = nc.dram_tensor("csi", [32, FCHUNK], fp32,
                           kind="ExternalOutput").ap()

    with tile.TileContext(nc) as tc:
        with (
            tc.tile_pool(name="io", bufs=4) as io_pool,
            tc.tile_pool(name="small", bufs=1) as small,
            tc.tile_pool(name="epool", bufs=4) as epool,
        ):
            # constants + frequency stack on the gpsimd (SWDGE) queue so the
            # sync/scalar HWDGE rings stay dedicated to ray data
            sel_sb = small.tile([128, 384], fp16, tag="sel")
            nc.gpsimd.dma_start(sel_sb[:], sel_d[:])
            csel_sb = small.tile([128, 512], fp16, tag="csel")
            nc.gpsimd.dma_start(csel_sb[:], csel_d[:])
            gs_sb = small.tile([128, F], fp16, tag="gs")
            nc.gpsimd.dma_start(gs_sb[:], gs_d[:])

            # ---- main loop: S accumulation over 256 fp16 slices ----
            sf_sb = small.tile([128, 128], fp32, tag="sf_sb")
            s_sb = small.tile([128, 128], fp16, tag="s_sb")
            total = N_TILE * N_SLICE
            with tc.tile_pool(name="spsum", bufs=1, space="PSUM") as spsum:
                banks = [spsum.tile([128, 128], fp32, tag=f"s{b}",
                                    name=f"sbank{b}") for b in range(NB)]
                seen = [False] * NB
                idx = 0
                for i in range(N_TILE):
                    rad_t = io_pool.tile([128, TILE_COLS], fp16, tag="rad")
                    att_t = io_pool.tile([128, TILE_COLS], fp16, tag="att")
                    if i == N_TILE - 1:
                        # halve the final loads so the tail MMs start sooner
                        hm = TILE_COLS // 2
                        nc.sync.dma_start(rad_t[:, 0:hm], rad_d[i, :, 0:hm])
                        nc.scalar.dma_start(att_t[:, 0:hm], att_d[i, :, 0:hm])
                        nc.sync.dma_start(rad_t[:, hm:], rad_d[i, :, hm:])
                        nc.scalar.dma_start(att_t[:, hm:], att_d[i, :, hm:])
                    else:
                        nc.sync.dma_start(rad_t[:], rad_d[i, :, :])
                        nc.scalar.dma_start(att_t[:], att_d[i, :, :])
                    for s in range(N_SLICE):
                        sl = slice(s * 128, (s + 1) * 128)
                        b = idx % NB
                        nc.tensor.matmul(banks[b][:], lhsT=rad_t[:, sl],
                                         rhs=att_t[:, sl], start=not seen[b],
                                         stop=(idx >= total - NB))
                        seen[b] = True
                        idx += 1
                # combine banks -> s_sb fp16 (one PSUM operand per DVE op)
                nc.vector.tensor_copy(sf_sb[:], banks[0][:])
                nc.vector.tensor_add(s_sb[:], sf_sb[:], banks[1][:])

            # ---- W build: 8 selector matmuls -> wps (64,64) PSUM ----
            whh = small.tile([64, 128], fp16, tag="whh")
            IA = sel_sb[:, 0:64]
            IB = sel_sb[:, 64:128]
            XA = sel_sb[:, 128:192]
            XB = sel_sb[:, 192:256]
            XNA = sel_sb[:, 256:320]
            XNB = sel_sb[:, 320:384]
            with tc.tile_pool(name="wpsum", bufs=1, space="PSUM") as wpsum:
                wps = wpsum.tile([64, 64], fp32, tag="wps")
                for lh, rc, st, sp in (
                    (IA, 0, True, False), (IB, 64, False, False),
                    (XA, 32, False, False), (XB, 96, False, True),
                ):
                    nc.tensor.matmul(wps[:, 0:32], lhsT=lh,
                                     rhs=s_sb[:, rc:rc + 32],
                                     start=st, stop=sp, skip_group_check=True)
                for lh, rc, st, sp in (
                    (XNA, 0, True, False), (XNB, 64, False, False),
                    (IA, 32, False, False), (IB, 96, False, True),
                ):
                    nc.tensor.matmul(wps[:, 32:64], lhsT=lh,
                                     rhs=s_sb[:, rc:rc + 32],
                                     start=st, stop=sp, skip_group_check=True)
                nc.scalar.copy(whh[:, 0:64], wps[:])
                nc.vector.tensor_copy(whh[:, 64:128], wps[:])

            # ---- phase 3: 16 chunks of 512 subcarriers ----
            csi_sb = small.tile([32, FCHUNK], fp32, tag="csi_sb")
            with (
                tc.tile_pool(name="tpsum", bufs=5, space="PSUM") as tpsum,
                tc.tile_pool(name="cpsum", bufs=1, space="PSUM") as cpsum,
            ):
                c_acc = cpsum.tile([32, FCHUNK], fp32, tag="c_acc")
                for ci in range(N_FCHUNK):
                    fs = slice(ci * FCHUNK, (ci + 1) * FCHUNK)
                    t_ps = tpsum.tile([128, FCHUNK], fp32, tag="t",
                                      name=f"t{ci}")
                    nc.tensor.matmul(t_ps[:], lhsT=whh[:],
                                     rhs=gs_sb[0:64, fs],
                                     start=True, stop=True)
                    e_sb = epool.tile([128, FCHUNK], fp16, tag="e",
                                      name=f"e{ci}")
                    if ci % 3 == 2:
                        # direct: DVE reads PSUM fp32 (1x mode)
                        nc.vector.tensor_mul(e_sb[:], gs_sb[:, fs], t_ps[:])
                    else:
                        # cast on ACT, then DVE fp16 TT at 2x
                        tc16 = epool.tile([128, FCHUNK], fp16, tag="tc",
                                          name=f"tc{ci}")
                        nc.scalar.copy(tc16[:], t_ps[:])
                        nc.vector.tensor_mul(e_sb[:], gs_sb[:, fs], tc16[:])
                    nc.tensor.matmul(c_acc[:],
                                     lhsT=csel_sb[:, 32 * ci:32 * ci + 32],
                                     rhs=e_sb[:], start=(ci == 0),
                                     stop=(ci == N_FCHUNK - 1),
                                     skip_group_check=True)
                nc.scalar.copy(csi_sb[:], c_acc[:])

            nc.sync.dma_start(out_d[:], csi_sb[:])

    nc.compile()
    return nc


def _pack_planar(arr, core):
    """Core's complex64 shard -> (N_TILE, 128, TILE_COLS) planar fp16."""
    sh = arr[core * DIR_PER_CORE:(core + 1) * DIR_PER_CORE]
    n = DIR_PER_CORE * K
    u = np.empty((n, 64), np.float16)
    u[:, :32] = sh.real.reshape(n, 32)
    u[:, 32:] = sh.imag.reshape(n, 32)
    t = u.reshape(N_TILE, N_SLICE, 2, 128, 64).transpose(0, 3, 1, 2, 4)
    return np.ascontiguousarray(t.reshape(N_TILE, 128, TILE_COLS))


def _build_gs(fbv):
    """(128, F) fp16 rows: vr(32), vi(32), vi(32), -vr(32)."""
    vr = np.ascontiguousarray(fbv.real.T)
    vi = np.ascontiguousarray(fbv.imag.T)
    return np.concatenate([vr, vi, vi, -vr], axis=0).astype(np.float16)


def _build_in_maps(attenuation_vectors, radiation_vectors,
                   frequency_basis_vectors):
    gs = _build_gs(frequency_basis_vectors)
    sel = _build_sel()
    csel = _build_csel()
    in_maps = []
    for c in range(N_CORES):
        in_maps.append({
            "rad": _pack_planar(radiation_vectors, c),
            "att": _pack_planar(attenuation_vectors, c),
            "gs": gs, "sel": sel, "csel": csel,
        })
    return in_maps


def kernel(attenuation_vectors, radiation_vectors, frequency_basis_vectors):
    from concourse.bass_utils import run_bass_kernel_spmd

    if "nc" not in _NC_CACHE:
        _NC_CACHE["nc"] = build_nc()
    nc = _NC_CACHE["nc"]

    in_maps = _build_in_maps(attenuation_vectors, radiation_vectors,
                             frequency_basis_vectors)
    res = run_bass_kernel_spmd(nc, in_maps, core_ids=list(range(N_CORES)))
    acc = np.zeros((32, FCHUNK), np.float64)
    for r in res.results:
        acc += r["csi"]
    re = acc[0::2].reshape(-1)
    im = acc[1::2].reshape(-1)
    return (re + 1j * im).astype(np.complex64)


# revision 6
# speedup vs baseline: 2.2489x; 1.1102x over previous
"""Trainium2 Bass kernel for LowRankRayTracer.

csi[f] = (delta_t/D) * v_f^T M v_f,  M = conj(rad)^T conj(att)  (R=32, complex)
contracted over N = D*K = 524288 rows.

v2 design (8 cores, ray-sharded, all-fp16 data path):
  - Host converts each core's ray shard to PLANAR fp16 rows u = [Re|Im] (64
    wide), 2-packed per partition: tile (128, 4096) x 8, so each 128-col slice
    is one lhsT/rhs pair contracting 256 rows.  fp16 halves HBM traffic vs the
    exact hi/lo fp32 split; quantization error ~5e-4 << 2e-2 gate.
  - 256 matmuls accumulate S (128x128 quadrants) into 2 PSUM banks.
  - W_real = [[Q,P],[P,-Q]] (Q=S_rr-S_ii, P=S_ri+S_ir, scale folded into
    selector consts) is built by 8 small selector matmuls straight into PSUM,
    then cast twice into whh = [W|W] (64x128 fp16).  Because
    T_imag = rot(T_real), one T = whh^T g matmul per chunk covers re+im.
  - e = g_stack .* T (g_stack rows: vr,vi,vi,-vr) via DVE (direct or
    ACT-cast+DVE-2x split to balance engines), then per-chunk selector
    matmuls accumulate all 16 chunk sums into ONE (32,512) PSUM tile:
    a single PSUM->SBUF copy + one 64 KiB output DMA.
"""

import numpy as np

D, K, R = 4096, 128, 32
F = 8192
N_CORES = 8
DIR_PER_CORE = D // N_CORES              # 512
N_TILE = 8                               # DMA tiles per tensor per core
TILE_COLS = 4096                         # fp16 cols per partition per tile
N_SLICE = TILE_COLS // 128               # 32 matmul slices per tile
NB = 2                                   # round-robin PSUM accumulator banks
S_SCALE = (200.0 / K) / D                # delta_t / num_directions (exact)
FCHUNK = 512
N_FCHUNK = F // FCHUNK                   # 16

_NC_CACHE = {}


def _build_sel():
    """(128, 384) fp16: C_IA|C_IB|C_XA|C_XB|-C_XA|-C_XB, scale folded in."""
    s = np.float16(S_SCALE)
    c = np.zeros((128, 384), np.float16)
    for p in range(64):
        c[p, p] = s            # C_IA
        c[64 + p, 64 + p] = s  # C_IB
    for i in range(32):
        c[i, 128 + 32 + i] = s       # C_XA
        c[32 + i, 128 + i] = -s
        c[64 + i, 192 + 32 + i] = s  # C_XB
        c[96 + i, 192 + i] = -s
    c[:, 256:320] = -c[:, 128:192]   # -C_XA
    c[:, 320:384] = -c[:, 192:256]   # -C_XB
    return c


def _build_csel():
    """(128, 512) fp16: chunk ci block (128,32) sums rows 0:64 -> col 2ci,
    rows 64:128 -> col 2ci+1."""
    c = np.zeros((128, 512), np.float16)
    for ci in range(N_FCHUNK):
        c[0:64, 32 * ci + 2 * ci] = 1.0
        c[64:128, 32 * ci + 2 * ci + 1] = 1.0
    return c


def build_nc():
    import concourse.bacc as bacc
    import concourse.mybir as mybir
    import concourse.tile as tile

    fp32 = mybir.dt.float32
    fp16 = mybir.dt.float16
    nc = bacc.Bacc(trn_type="TRN2", target_bir_lowering=False, debug=False)

    rad_d = nc.dram_tensor("rad", [N_TILE, 128, TILE_COLS], fp16,
                           kind="ExternalInput").ap()
    att_d = nc.dram_tensor("att", [N_TILE, 128, TILE_COLS], fp16,
                           kind="ExternalInput").ap()
    gs_d = nc.dram_tensor("gs", [128, F], fp16, kind="ExternalInput").ap()
    sel_d = nc.dram_tensor("sel", [128, 384], fp16, kind="ExternalInput").ap()
    csel_d = nc.dram_tensor("csel", [128, 512], fp16,
                            kind="ExternalInput").ap()
    out_d = nc.dram_tensor("csi", [32, FCHUNK], fp32,
                           kind="ExternalOutput").ap()

    with tile.TileContext(nc) as tc:
        with (
            tc.tile_pool(name="io", bufs=4) as io_pool,
            tc.tile_pool(name="small", bufs=1) as small,
            tc.tile_pool(name="epool", bufs=4) as epool,
        ):
            # constants + frequency stack on the gpsimd (SWDGE) queue so the
            # sync/scalar HWDGE rings stay dedicated to ray data
            sel_sb = small.tile([128, 384], fp16, tag="sel")
            nc.gpsimd.dma_start(sel_sb[:], sel_d[:])
            csel_sb = small.tile([128, 512], fp16, tag="csel")
            nc.gpsimd.dma_start(csel_sb[:], csel_d[:])
            gs_sb = small.tile([128, F], fp16, tag="gs")
            nc.gpsimd.dma_start(gs_sb[:], gs_d[:])

            # ---- main loop: S accumulation over 256 fp16 slices ----
            sf_sb = small.tile([128, 128], fp32, tag="sf_sb")
            s_sb = small.tile([128, 128], fp16, tag="s_sb")
            total = N_TILE * N_SLICE
            # warm-keeper dummy matmuls: fill PE idle during DMA-bound tile
            # gaps so the HAM clock gate stays at 8/8 into the tail phase
            DUMMIES = {3: 24, 4: 34, 5: 34, 6: 34}
            with (
                tc.tile_pool(name="spsum", bufs=1, space="PSUM") as spsum,
                tc.tile_pool(name="dpsum", bufs=1, space="PSUM") as dpsum,
            ):
                dummy_ps = dpsum.tile([128, 256], fp32, tag="dummy")

                def warm(n):
                    for _ in range(n):
                        nc.tensor.matmul(dummy_ps[:], lhsT=gs_sb[:, 0:128],
                                         rhs=gs_sb[:, 0:256], start=True,
                                         stop=True, skip_group_check=True)

                banks = [spsum.tile([128, 128], fp32, tag=f"s{b}",
                                    name=f"sbank{b}") for b in range(NB)]
                seen = [False] * NB
                idx = 0
                for i in range(N_TILE):
                    rad_t = io_pool.tile([128, TILE_COLS], fp16, tag="rad")
                    att_t = io_pool.tile([128, TILE_COLS], fp16, tag="att")
                    if i == N_TILE - 1:
                        # halve the final loads so the tail MMs start sooner
                        hm = TILE_COLS // 2
                        nc.sync.dma_start(rad_t[:, 0:hm], rad_d[i, :, 0:hm])
                        nc.scalar.dma_start(att_t[:, 0:hm], att_d[i, :, 0:hm])
                        nc.sync.dma_start(rad_t[:, hm:], rad_d[i, :, hm:])
                        nc.scalar.dma_start(att_t[:, hm:], att_d[i, :, hm:])
                    else:
                        nc.sync.dma_start(rad_t[:], rad_d[i, :, :])
                        nc.scalar.dma_start(att_t[:], att_d[i, :, :])
                    n_sl = N_SLICE if i < N_TILE - 1 else N_SLICE // 2
                    for s in range(n_sl):
                        sl = slice(s * 128, (s + 1) * 128)
                        b = idx % NB
                        nc.tensor.matmul(banks[b][:], lhsT=rad_t[:, sl],
                                         rhs=att_t[:, sl], start=not seen[b],
                                         stop=(idx >= total - NB))
                        seen[b] = True
                        idx += 1
                    if i == N_TILE - 1:
                        warm(10)
                        for s in range(N_SLICE // 2, N_SLICE):
                            sl = slice(s * 128, (s + 1) * 128)
                            b = idx % NB
                            nc.tensor.matmul(banks[b][:], lhsT=rad_t[:, sl],
                                             rhs=att_t[:, sl],
                                             start=not seen[b],
                                             stop=(idx >= total - NB))
                            seen[b] = True
                            idx += 1
                        warm(6)
                    else:
                        warm(DUMMIES.get(i, 0))
                # combine banks -> s_sb fp16 (one PSUM operand per DVE op)
                nc.vector.tensor_copy(sf_sb[:], banks[0][:])
                nc.vector.tensor_add(s_sb[:], sf_sb[:], banks[1][:])

            # ---- W build: 8 selector matmuls -> wps (64,64) PSUM ----
            whh = small.tile([64, 128], fp16, tag="whh")
            IA = sel_sb[:, 0:64]
            IB = sel_sb[:, 64:128]
            XA = sel_sb[:, 128:192]
            XB = sel_sb[:, 192:256]
            XNA = sel_sb[:, 256:320]
            XNB = sel_sb[:, 320:384]
            with tc.tile_pool(name="wpsum", bufs=1, space="PSUM") as wpsum:
                wps = wpsum.tile([64, 64], fp32, tag="wps")
                for lh, rc, st, sp in (
                    (IA, 0, True, False), (IB, 64, False, False),
                    (XA, 32, False, False), (XB, 96, False, True),
                ):
                    nc.tensor.matmul(wps[:, 0:32], lhsT=lh,
                                     rhs=s_sb[:, rc:rc + 32],
                                     start=st, stop=sp, skip_group_check=True)
                for lh, rc, st, sp in (
                    (XNA, 0, True, False), (XNB, 64, False, False),
                    (IA, 32, False, False), (IB, 96, False, True),
                ):
                    nc.tensor.matmul(wps[:, 32:64], lhsT=lh,
                                     rhs=s_sb[:, rc:rc + 32],
                                     start=st, stop=sp, skip_group_check=True)
                nc.vector.tensor_copy(whh[:, 0:64], wps[:])
                nc.vector.tensor_copy(whh[:, 64:128], wps[:])

            # ---- phase 3: 8 super-chunks of 1024 subcarriers ----
            # per super-chunk: 2 T matmuls (N=512), one 1024-wide e-mul
            # (direct DVE-from-PSUM or ACT-cast + DVE fp16 2x), 2 c matmuls
            # lagged one super-chunk so the in-order PE queue never blocks
            # on a pending e tile.
            csi_sb = small.tile([32, FCHUNK], fp32, tag="csi_sb")
            N_SC = 8
            SC = 2 * FCHUNK
            DIRECT = {2, 5}
            with (
                tc.tile_pool(name="tpsum", bufs=3, space="PSUM") as tpsum,
                tc.tile_pool(name="cpsum", bufs=1, space="PSUM") as cpsum,
            ):
                c_acc = cpsum.tile([32, FCHUNK], fp32, tag="c_acc")
                e_tiles = {}

                def emit_c(sc):
                    for h in range(2):
                        k = 2 * sc + h
                        nc.tensor.matmul(
                            c_acc[:], lhsT=csel_sb[:, 32 * k:32 * k + 32],
                            rhs=e_tiles[sc][:, FCHUNK * h:FCHUNK * (h + 1)],
                            start=(k == 0), stop=(k == 2 * N_SC - 1),
                            skip_group_check=True)

                for sc in range(N_SC):
                    fs = slice(sc * SC, (sc + 1) * SC)
                    t_ps = tpsum.tile([128, SC], fp32, tag="t", name=f"t{sc}")
                    for h in range(2):
                        hs = slice(FCHUNK * h, FCHUNK * (h + 1))
                        nc.tensor.matmul(t_ps[:, hs], lhsT=whh[:],
                                         rhs=gs_sb[0:64, sc * SC + FCHUNK * h:
                                                    sc * SC + FCHUNK * (h + 1)],
                                         start=True, stop=True,
                                         skip_group_check=True)
                    e_sb = epool.tile([128, SC], fp16, tag="e", name=f"e{sc}")
                    e_tiles[sc] = e_sb
                    if sc in DIRECT:
                        nc.vector.tensor_mul(e_sb[:], gs_sb[:, fs], t_ps[:])
                    else:
                        tc16 = epool.tile([128, SC], fp16, tag="tc",
                                          name=f"tc{sc}")
                        nc.scalar.copy(tc16[:], t_ps[:])
                        nc.vector.tensor_mul(e_sb[:], gs_sb[:, fs], tc16[:])
                    if sc >= 2:
                        emit_c(sc - 2)
                emit_c(N_SC - 2)
                emit_c(N_SC - 1)
                nc.scalar.copy(csi_sb[:], c_acc[:])

            nc.sync.dma_start(out_d[:], csi_sb[:])

    nc.compile()
    return nc


def _pack_planar(arr, core):
    """Core's complex64 shard -> (N_TILE, 128, TILE_COLS) planar fp16."""
    sh = arr[core * DIR_PER_CORE:(core + 1) * DIR_PER_CORE]
    n = DIR_PER_CORE * K
    u = np.empty((n, 64), np.float16)
    u[:, :32] = sh.real.reshape(n, 32)
    u[:, 32:] = sh.imag.reshape(n, 32)
    t = u.reshape(N_TILE, N_SLICE, 2, 128, 64).transpose(0, 3, 1, 2, 4)
    return np.ascontiguousarray(t.reshape(N_TILE, 128, TILE_COLS))


def _build_gs(fbv):
    """(128, F) fp16 rows: vr(32), vi(32), vi(32), -vr(32)."""
    vr = np.ascontiguousarray(fbv.real.T)
    vi = np.ascontiguousarray(fbv.imag.T)
    return np.concatenate([vr, vi, vi, -vr], axis=0).astype(np.float16)


def _build_in_maps(attenuation_vectors, radiation_vectors,
                   frequency_basis_vectors):
    gs = _build_gs(frequency_basis_vectors)
    sel = _build_sel()
    csel = _build_csel()
    in_maps = []
    for c in range(N_CORES):
        in_maps.append({
            "rad": _pack_planar(radiation_vectors, c),
            "att": _pack_planar(attenuation_vectors, c),
            "gs": gs, "sel": sel, "csel": csel,
        })
    return in_maps


def kernel(attenuation_vectors, radiation_vectors, frequency_basis_vectors):
    from concourse.bass_utils import run_bass_kernel_spmd

    if "nc" not in _NC_CACHE:
        _NC_CACHE["nc"] = build_nc()
    nc = _NC_CACHE["nc"]

    in_maps = _build_in_maps(attenuation_vectors, radiation_vectors,
                             frequency_basis_vectors)
    res = run_bass_kernel_spmd(nc, in_maps, core_ids=list(range(N_CORES)))
    acc = np.zeros((32, FCHUNK), np.float64)
    for r in res.results:
        acc += r["csi"]
    re = acc[0::2].reshape(-1)
    im = acc[1::2].reshape(-1)
    return (re + 1j * im).astype(np.complex64)


# revision 9
# speedup vs baseline: 2.4209x; 1.0765x over previous
"""Trainium2 Bass kernel for LowRankRayTracer.

csi[f] = (delta_t/D) * v_f^T M v_f,  M = conj(rad)^T conj(att)  (R=32, complex)
contracted over N = D*K = 524288 rows.

v4 design (8 cores, ray-sharded, all-fp16 data path):
  - Host converts each core's ray shard to PLANAR fp16 rows u = [Re|Im] (64
    wide), 2-packed per partition: tile (128, 4096) x 8, so each 128-col slice
    is one lhsT/rhs pair contracting 256 rows.  fp16 halves HBM traffic vs the
    exact hi/lo fp32 split; quantization error ~6e-4 << 2e-2 gate.
  - 256 matmuls accumulate S (128x128 quadrants) in PSUM; split A (tiles 0-6)
    / B (tile 7) so the A-part of the S->W_real epilogue hides under tile-7's
    DMA.  W_real = [[Q,P],[P,-Q]] (Q=S_rr-S_ii, P=S_ri+S_ir, scale folded into
    selector consts) is built by 16 small selector matmuls accumulating into
    one PSUM tile, cast twice into whh = [W|W] (64x128 fp16): because
    T_imag = rot(T_real), one T = whh^T g matmul per chunk covers re+im.
  - Warm-keeper dummy matmuls fill PE idle during the DMA-bound stretch so
    the HAM clock gate stays at 8/8 (2.4 GHz) through the tail.
  - Phase 3 in 8 super-chunks of 1024 subcarriers: 2 T matmuls, one 1024-wide
    e = g_stack .* T (g_stack rows: vr,vi,vi,-vr; direct DVE-from-PSUM or
    ACT-cast + DVE fp16 2x, balanced), 2 c matmuls lagged two super-chunks so
    the in-order PE queue never blocks; all chunk sums land in two (16,512)
    PSUM tiles so the finalize is 2 copies + 2 output DMAs, first pair
    overlapped with the second half of the pipeline.
"""

import numpy as np

D, K, R = 4096, 128, 32
F = 8192
N_CORES = 8
DIR_PER_CORE = D // N_CORES              # 512
N_TILE = 8                               # DMA tiles per tensor per core
TILE_COLS = 4096                         # fp16 cols per partition per tile
N_SLICE = TILE_COLS // 128               # 32 matmul slices per tile
S_SCALE = (200.0 / K) / D                # delta_t / num_directions (exact)
FCHUNK = 512
N_SC = 8                                 # phase-3 super-chunks
SC = 2 * FCHUNK                          # 1024 subcarriers per super-chunk

_NC_CACHE = {}


def _build_sel():
    """(128, 384) fp16: C_IA|C_IB|C_XA|C_XB|-C_XA|-C_XB, scale folded in."""
    s = np.float16(S_SCALE)
    c = np.zeros((128, 384), np.float16)
    for p in range(64):
        c[p, p] = s            # C_IA
        c[64 + p, 64 + p] = s  # C_IB
    for i in range(32):
        c[i, 128 + 32 + i] = s       # C_XA
        c[32 + i, 128 + i] = -s
        c[64 + i, 192 + 32 + i] = s  # C_XB
        c[96 + i, 192 + i] = -s
    c[:, 256:320] = -c[:, 128:192]   # -C_XA
    c[:, 320:384] = -c[:, 192:256]   # -C_XB
    return c


def _build_csel():
    """(128, 256) fp16: chunk k block (128,16) sums rows 0:64 -> local col
    2*(k%8), rows 64:128 -> 2*(k%8)+1 (two 16-row accumulators)."""
    c = np.zeros((128, 256), np.float16)
    for k in range(16):
        c[0:64, 16 * k + 2 * (k % 8)] = 1.0
        c[64:128, 16 * k + 2 * (k % 8) + 1] = 1.0
    return c


def build_nc():
    import concourse.bacc as bacc
    import concourse.mybir as mybir
    import concourse.tile as tile

    fp32 = mybir.dt.float32
    fp16 = mybir.dt.float16
    nc = bacc.Bacc(trn_type="TRN2", target_bir_lowering=False, debug=False)

    rad_d = nc.dram_tensor("rad", [N_TILE, 128, TILE_COLS], fp16,
                           kind="ExternalInput").ap()
    att_d = nc.dram_tensor("att", [N_TILE, 128, TILE_COLS], fp16,
                           kind="ExternalInput").ap()
    gs_d = nc.dram_tensor("gs", [128, F], fp16, kind="ExternalInput").ap()
    sel_d = nc.dram_tensor("sel", [128, 384], fp16, kind="ExternalInput").ap()
    csel_d = nc.dram_tensor("csel", [128, 256], fp16,
                            kind="ExternalInput").ap()
    out_d = nc.dram_tensor("csi", [32, FCHUNK], fp32,
                           kind="ExternalOutput").ap()

    with tile.TileContext(nc) as tc:
        with (
            tc.tile_pool(name="io", bufs=4) as io_pool,
            tc.tile_pool(name="small", bufs=1) as small,
            tc.tile_pool(name="epool", bufs=4) as epool,
        ):
            # constants + frequency stack on the gpsimd (SWDGE) queue so the
            # sync/scalar HWDGE rings stay dedicated to ray data
            sel_sb = small.tile([128, 384], fp16, tag="sel")
            nc.gpsimd.dma_start(sel_sb[:], sel_d[:])
            csel_sb = small.tile([128, 256], fp16, tag="csel")
            nc.gpsimd.dma_start(csel_sb[:], csel_d[:])
            gs_sb = small.tile([128, F], fp16, tag="gs")
            nc.gpsimd.dma_start(gs_sb[:], gs_d[:])

            sfa = small.tile([128, 128], fp32, tag="sfa")
            s_sba = small.tile([128, 128], fp16, tag="s_sba")
            sfb = small.tile([128, 128], fp32, tag="sfb")
            s_sbb = small.tile([128, 128], fp16, tag="s_sbb")
            whh = small.tile([64, 128], fp16, tag="whh")
            IA = sel_sb[:, 0:64]
            IB = sel_sb[:, 64:128]
            XA = sel_sb[:, 128:192]
            XB = sel_sb[:, 192:256]
            XNA = sel_sb[:, 256:320]
            XNB = sel_sb[:, 320:384]
            # selector matmul plan: (lhsT, s_sb col, dest col block)
            WPLAN0 = ((IA, 0), (IB, 64), (XA, 32), (XB, 96))
            WPLAN1 = ((XNA, 0), (XNB, 64), (IA, 32), (IB, 96))

            DUMMIES = {3: 24, 4: 30, 5: 30, 6: 12}
            with tc.tile_pool(name="mpsum", bufs=1, space="PSUM") as mpsum:
                dummy_ps = mpsum.tile([128, 256], fp32, tag="dummy")
                wps = mpsum.tile([64, 64], fp32, tag="wps")

                def warm(n):
                    for _ in range(n):
                        nc.tensor.matmul(dummy_ps[:], lhsT=gs_sb[:, 0:128],
                                         rhs=gs_sb[:, 0:256], start=True,
                                         stop=True, skip_group_check=True)

                def wmm(s_sb, dst, plan, start, stop):
                    for j, (lh, rc) in enumerate(plan):
                        nc.tensor.matmul(wps[:, dst:dst + 32], lhsT=lh,
                                         rhs=s_sb[:, rc:rc + 32],
                                         start=(start and j == 0),
                                         stop=(stop and j == len(plan) - 1),
                                         skip_group_check=True)

                banksA = [mpsum.tile([128, 128], fp32, tag=f"sa{b}",
                                     name=f"sa{b}") for b in range(2)]
                idx = 0
                for i in range(N_TILE):
                    if i < N_TILE - 1:
                        rad_t = io_pool.tile([128, TILE_COLS], fp16,
                                             tag="rad")
                        att_t = io_pool.tile([128, TILE_COLS], fp16,
                                             tag="att")
                        nc.sync.dma_start(rad_t[:], rad_d[i, :, :])
                        nc.scalar.dma_start(att_t[:], att_d[i, :, :])
                    else:
                        # tile 7 in quarters so MMs chase the DMA tail
                        rad_t = io_pool.tile([128, TILE_COLS], fp16,
                                             tag="rad")
                        att_t = io_pool.tile([128, TILE_COLS], fp16,
                                             tag="att")
                        qc = TILE_COLS // 4
                        for q in range(4):
                            qs = slice(q * qc, (q + 1) * qc)
                            nc.sync.dma_start(rad_t[:, qs], rad_d[7, :, qs])
                            nc.scalar.dma_start(att_t[:, qs], att_d[7, :, qs])
                    for s in range(N_SLICE):
                        sl = slice(s * 128, (s + 1) * 128)
                        nc.tensor.matmul(banksA[idx % 2][:],
                                         lhsT=rad_t[:, sl], rhs=att_t[:, sl],
                                         start=(idx < 2),
                                         stop=(idx >= 8 * N_SLICE - 2))
                        idx += 1
                    warm(DUMMIES.get(i, 0))
                warm(6)
                nc.vector.tensor_copy(sfa[:], banksA[0][:])
                nc.vector.tensor_add(s_sba[:], sfa[:], banksA[1][:])
                wmm(s_sba, 0, WPLAN0, start=True, stop=True)
                wmm(s_sba, 32, WPLAN1, start=True, stop=True)
                warm(4)
                nc.vector.tensor_copy(whh[:, 0:64], wps[:])
                nc.vector.tensor_copy(whh[:, 64:128], wps[:])

            # ---- phase 3: 8 super-chunks of 1024 subcarriers ----
            csiA = small.tile([16, FCHUNK], fp32, tag="csiA")
            csiB = small.tile([16, FCHUNK], fp32, tag="csiB")
            DIRECT = {2, 5}
            with (
                tc.tile_pool(name="tpsum", bufs=3, space="PSUM") as tpsum,
                tc.tile_pool(name="cpsum", bufs=1, space="PSUM") as cpsum,
            ):
                c_accA = cpsum.tile([16, FCHUNK], fp32, tag="c_accA")
                c_accB = cpsum.tile([16, FCHUNK], fp32, tag="c_accB")
                e_tiles = {}

                def emit_c(sc):
                    c_acc = c_accA if sc < 4 else c_accB
                    for h in range(2):
                        k = 2 * sc + h
                        nc.tensor.matmul(
                            c_acc[:], lhsT=csel_sb[:, 16 * k:16 * k + 16],
                            rhs=e_tiles[sc][:, FCHUNK * h:FCHUNK * (h + 1)],
                            start=(k % 8 == 0), stop=(k % 8 == 7),
                            skip_group_check=True)

                for sc in range(N_SC):
                    fs = slice(sc * SC, (sc + 1) * SC)
                    t_ps = tpsum.tile([128, SC], fp32, tag="t", name=f"t{sc}")
                    for h in range(2):
                        hs = slice(FCHUNK * h, FCHUNK * (h + 1))
                        nc.tensor.matmul(t_ps[:, hs], lhsT=whh[:],
                                         rhs=gs_sb[0:64, sc * SC + FCHUNK * h:
                                                    sc * SC + FCHUNK * (h + 1)],
                                         start=True, stop=True,
                                         skip_group_check=True)
                    e_sb = epool.tile([128, SC], fp16, tag="e", name=f"e{sc}")
                    e_tiles[sc] = e_sb
                    if sc in DIRECT:
                        nc.vector.tensor_mul(e_sb[:], gs_sb[:, fs], t_ps[:])
                    else:
                        tc16 = epool.tile([128, SC], fp16, tag="tc",
                                          name=f"tc{sc}")
                        nc.scalar.copy(tc16[:], t_ps[:])
                        nc.vector.tensor_mul(e_sb[:], gs_sb[:, fs], tc16[:])
                    if sc >= 2:
                        emit_c(sc - 2)
                    if sc == 7:
                        # first-half finalize overlaps the last super-chunks
                        nc.scalar.copy(csiA[:], c_accA[:])
                        nc.sync.dma_start(out_d[0:16, :], csiA[:])
                emit_c(N_SC - 2)
                emit_c(N_SC - 1)
                nc.scalar.copy(csiB[:], c_accB[:])

            nc.sync.dma_start(out_d[16:32, :], csiB[:])

    nc.compile()
    return nc


def _pack_planar(arr, core):
    """Core's complex64 shard -> (N_TILE, 128, TILE_COLS) planar fp16."""
    sh = arr[core * DIR_PER_CORE:(core + 1) * DIR_PER_CORE]
    n = DIR_PER_CORE * K
    u = np.empty((n, 64), np.float16)
    u[:, :32] = sh.real.reshape(n, 32)
    u[:, 32:] = sh.imag.reshape(n, 32)
    t = u.reshape(N_TILE, N_SLICE, 2, 128, 64).transpose(0, 3, 1, 2, 4)
    return np.ascontiguousarray(t.reshape(N_TILE, 128, TILE_COLS))


def _build_gs(fbv):
    """(128, F) fp16 rows: vr(32), vi(32), vi(32), -vr(32)."""
    vr = np.ascontiguousarray(fbv.real.T)
    vi = np.ascontiguousarray(fbv.imag.T)
    return np.concatenate([vr, vi, vi, -vr], axis=0).astype(np.float16)


def _build_in_maps(attenuation_vectors, radiation_vectors,
                   frequency_basis_vectors):
    gs = _build_gs(frequency_basis_vectors)
    sel = _build_sel()
    csel = _build_csel()
    in_maps = []
    for c in range(N_CORES):
        in_maps.append({
            "rad": _pack_planar(radiation_vectors, c),
            "att": _pack_planar(attenuation_vectors, c),
            "gs": gs, "sel": sel, "csel": csel,
        })
    return in_maps


def kernel(attenuation_vectors, radiation_vectors, frequency_basis_vectors):
    from concourse.bass_utils import run_bass_kernel_spmd

    if "nc" not in _NC_CACHE:
        _NC_CACHE["nc"] = build_nc()
    nc = _NC_CACHE["nc"]

    in_maps = _build_in_maps(attenuation_vectors, radiation_vectors,
                             frequency_basis_vectors)
    res = run_bass_kernel_spmd(nc, in_maps, core_ids=list(range(N_CORES)))
    acc = np.zeros((32, FCHUNK), np.float64)
    for r in res.results:
        acc += r["csi"]
    re = acc[0::2].reshape(-1)
    im = acc[1::2].reshape(-1)
    return (re + 1j * im).astype(np.complex64)
